# revision 20
# baseline (speedup 1.0000x reference)
"""Trainium2 Bass kernel for nn_CaptioningRNN (attention LSTM over T=64).

Data-parallel over the batch: N=256 samples split across 8 NeuronCores
(32 samples/core), weights replicated, no collectives.

Per-core design (v3 — fully fused step loop):
  - No xproj prepass: x_t @ Wx accumulates into the same PSUM strips as
    h @ Wh and the attention term, so there is no DRAM scratch round trip
    and the TensorEngine stays dense (HAM stays warm).
  - Single ACT table set (exp_and_others): sigmoid is computed as
    0.5*(1+tanh(x/2)) so the per-step Exp (softmax) and Tanh (gates) never
    force an activation-table reload.
  - P precompute: P[n,k,:] = A[n,:,k] @ Wattn + b once; since softmax
    weights sum to 1, folding b into P applies the bias exactly.
  - Per step: scores via cross-sample matmuls + masked diag reduce;
    softmax (no max-subtract — scores are O(1) bounded); w expanded to the
    (k, n_g) block-diagonal stationary via one-hot matmul + broadcast mask
    mul; gates = h@Wh + x_t@Wx + sum_k w_k P_k in 2 column-tiled PSUM
    strips; strips transposed on PE; cell math in h-on-partition space.
  - Output written transposed [t, h, n]; host reassembles to (N, T, H).
"""

import numpy as np
import ml_dtypes

import concourse.bacc as bacc
import concourse.mybir as mybir
from concourse import bass_utils
from concourse.tile import TileContext

F32, BF16 = mybir.dt.float32, mybir.dt.bfloat16
AF = mybir.ActivationFunctionType
ALU = mybir.AluOpType
AX = mybir.AxisListType
BF = ml_dtypes.bfloat16

N, T, D, H = 256, 64, 1024, 1024
NCORES = 8
NL = N // NCORES          # 32 samples per core
HC = 8                    # 128-row chunks of D/H
G, GS = 4, 8              # sample groups of 8 (for the (k, n_g) 128-partition layout)
H4 = 4 * H                # 4096 gate columns

_built = None


def _consts():
    # E16[k', k] one-hot: expands wT rows onto the 128-partition (k, n_g) axis.
    e16 = np.zeros((16, 128), dtype=BF)
    for k in range(16):
        e16[k, 8 * k : 8 * k + 8] = 1
    # M32R[p, 128 g + 32 rep + m] = (m % 8 == p % 8) & (m // 8 == g):
    # block-diagonal mask producing masked = w[m, k(p)] only for group-g
    # samples, replicated over the 4 column-tile strips.
    p = np.arange(128)[:, None]
    m = np.arange(32)[None, :]
    m32r = np.zeros((128, 512), dtype=BF)
    for g in range(4):
        blk = ((m % 8 == p % 8) & (m // 8 == g)).astype(BF)
        for rep in range(4):
            m32r[:, 128 * g + 32 * rep : 128 * g + 32 * rep + 32] = blk
    # Mdiag4[32 j + n, 32 k + n'] = (n == n') / 32: extracts the diagonal of
    # the col-tiled cross-sample score partials and applies the 1/sqrt(H)
    # softmax scale (same pattern for each of the 4 partition tiles).
    md = np.zeros((32, 512), dtype=np.float32)
    n_ = np.arange(32)
    for k in range(16):
        md[n_, 32 * k + n_] = 1.0 / 32.0
    # uTh holds 2h, so the score scale absorbs an extra 1/2 (1/64 total)
    md4 = np.tile(md, (4, 1)) * 0.5
    # E32[p, m] = (p % 32 == m): sums the 4 col-tiled score partials (f32
    # stationary so the tiny N=16 matmul needs no cast of its moving operand).
    e32 = np.zeros((128, 32), dtype=np.float32)
    e32[np.arange(128), np.arange(128) % 32] = 1
    # row-0 selector for the rank-1 bias accumulation into P.
    er0 = np.zeros((128, 128), dtype=BF)
    er0[0, :] = 1
    return e16, m32r, md4, e32, er0


def _build_nc(t_steps=T):
    nc = bacc.Bacc(trn_type="TRN2", target_bir_lowering=False, debug=False)

    # xTs[t, p, c, n] = x[n, t, 128 c + p] (bf16) — one contiguous DMA/step
    ap_xTs = nc.dram_tensor("xTs", [T, 128, HC * NL], BF16, kind="ExternalInput").ap()
    ap_Asc = nc.dram_tensor("Asc", [H, 512], BF16, kind="ExternalInput").ap()
    ap_Wx = nc.dram_tensor("Wx", [D, H4], BF16, kind="ExternalInput").ap()
    ap_Wh = nc.dram_tensor("Wh", [H, H4], BF16, kind="ExternalInput").ap()
    ap_Wattn = nc.dram_tensor("Wattn", [H, H4], BF16, kind="ExternalInput").ap()
    ap_brow = nc.dram_tensor("brow", [1, H4], BF16, kind="ExternalInput").ap()
    outT = nc.dram_tensor("outT", [T, H, NL], F32, kind="ExternalOutput").ap()

    e16_np, m32r_np, md4_np, e32_np, er0_np = _consts()
    eye_d = nc.inline_tensor(np.eye(128, dtype=np.float32), "c_eye")
    e16_d = nc.inline_tensor(e16_np, "c_e16")
    m32r_d = nc.inline_tensor(m32r_np, "c_m32r")
    md4_d = nc.inline_tensor(md4_np, "c_mdiag4")
    e32_d = nc.inline_tensor(e32_np, "c_e32")
    er0_d = nc.inline_tensor(er0_np, "c_er0")

    q4 = lambda ap: ap.rearrange("p (q c) -> p q c", q=4)

    with TileContext(nc) as tc:
        with tc.tile_pool(name="pers", bufs=1) as pers:
            Wh_sb = pers.tile([128, HC * H4], BF16, tag="Wh")
            Wx_sb = pers.tile([128, HC * H4], BF16, tag="Wx")
            Asc_sb = pers.tile([128, HC * 512], BF16, tag="Asc")
            P_sb = pers.tile([128, G * H4], BF16, tag="P")
            uTh = pers.tile([128, HC * NL], BF16, tag="uTh")
            cT = pers.tile([128, 256], F32, tag="cT")
            eye = pers.tile([128, 128], F32, tag="eye")
            E16 = pers.tile([16, 128], BF16, tag="E16")
            M32R = pers.tile([128, 512], BF16, tag="M32R")
            Mdiag4 = pers.tile([128, 512], F32, tag="Mdiag4")
            E32 = pers.tile([128, 32], F32, tag="E32")
            ER0 = pers.tile([128, 128], BF16, tag="ER0")
            wsq = pers.tile([32, 32], BF16, tag="wsq")

            # Asc + the first Wattn blocks are what phase B needs — issue
            # those DMAs before the big Wh/Wx loads so the P matmuls can
            # start early (the Wh/Wx weights are only needed at step 0).
            for c in range(HC):
                nc.sync.dma_start(
                    Asc_sb[:, c * 512 : (c + 1) * 512],
                    ap_Asc[128 * c : 128 * (c + 1), :],
                )
            nc.sync.dma_start(eye[:], eye_d.ap()[:])
            nc.sync.dma_start(E16[:], e16_d.ap()[:])
            nc.sync.dma_start(M32R[:], m32r_d.ap()[:])
            nc.sync.dma_start(Mdiag4[:], md4_d.ap()[:])
            nc.sync.dma_start(E32[:], e32_d.ap()[:])
            nc.sync.dma_start(ER0[:], er0_d.ap()[:])
            nc.gpsimd.memset(wsq[:], 0.0)

            # ------------- phase B: P precompute (+bias) + h0/c0 init -------------
            with tc.tile_pool(name="php1", bufs=1) as php1, \
                 tc.tile_pool(name="php", bufs=2) as php, \
                 tc.tile_pool(name="psP", bufs=2, space="PSUM") as psP:
                for c in range(HC):
                    h0s = php.tile([128, 32], F32, tag="h0s")
                    nc.vector.tensor_reduce(
                        h0s[:],
                        Asc_sb[:, c * 512 : (c + 1) * 512].rearrange(
                            "p (k n) -> p n k", k=16
                        ),
                        axis=AX.X,
                        op=ALU.add,
                    )
                    # cT holds C2 = 2*c and uTh holds 2*h throughout (the cell
                    # update keeps the doubled scale; tanh reads with scale=0.5
                    # and the host pre-scales Wh by 1/2)
                    nc.vector.tensor_scalar_mul(
                        cT[:, 32 * c : 32 * (c + 1)], h0s[:], 1.0 / 8.0
                    )
                    nc.vector.tensor_scalar_mul(
                        uTh[:, 32 * c : 32 * (c + 1)], h0s[:], 1.0 / 8.0
                    )
                # contiguous staging of the group-selected A columns so the
                # matmul stationary operand has a single free dim
                Ag = php1.tile([128, G * HC * 128], BF16, tag="Ag")
                for g in range(G):
                    for c in range(HC):
                        nc.vector.tensor_copy(
                            Ag[:, (g * HC + c) * 128 : (g * HC + c) * 128 + 128],
                            Asc_sb[:, c * 512 : (c + 1) * 512].rearrange(
                                "p (k n) -> p k n", k=16
                            )[:, :, GS * g : GS * (g + 1)],
                        )
                bmv = php1.tile([128, 512], BF16, tag="bmv")
                nc.gpsimd.memset(bmv[:], 0.0)
                for blk in range(8):
                    Wab = php.tile([128, HC * 512], BF16, tag="Wab")
                    for c in range(HC):
                        nc.sync.dma_start(
                            Wab[:, c * 512 : (c + 1) * 512],
                            ap_Wattn[128 * c : 128 * (c + 1), 512 * blk : 512 * (blk + 1)],
                        )
                    if blk == 1:
                        # big Wh/Wx loads issued after the first Wattn blocks:
                        # they are only needed once the recurrence starts
                        for c in range(HC):
                            nc.sync.dma_start(
                                Wh_sb[:, c * H4 : (c + 1) * H4],
                                ap_Wh[128 * c : 128 * (c + 1), :],
                            )
                            nc.sync.dma_start(
                                Wx_sb[:, c * H4 : (c + 1) * H4],
                                ap_Wx[128 * c : 128 * (c + 1), :],
                            )
                    bsl = php.tile([1, 512], BF16, tag="bsl")
                    nc.sync.dma_start(bsl[:], ap_brow[0:1, 512 * blk : 512 * (blk + 1)])
                    nc.vector.tensor_copy(bmv[0:1, :], bsl[:])
                    for g in range(G):
                        psp = psP.tile([128, 512], F32, tag="psp")
                        for c in range(HC):
                            nc.tensor.matmul(
                                psp[:],
                                Ag[:, (g * HC + c) * 128 : (g * HC + c) * 128 + 128],
                                Wab[:, c * 512 : (c + 1) * 512],
                                start=(c == 0),
                                stop=False,
                            )
                        # rank-1 accumulation: adds b[blk cols] to every row
                        # (softmax weights sum to 1, so this applies +b exactly)
                        nc.tensor.matmul(
                            psp[:], ER0[:], bmv[:], start=False, stop=True
                        )
                        nc.vector.tensor_copy(
                            P_sb[:, g * H4 + 512 * blk : g * H4 + 512 * (blk + 1)],
                            psp[:],
                        )

            # ---------------------- phase C: recurrence ----------------------
            with tc.tile_pool(name="wrk", bufs=2) as wrk, \
                 tc.tile_pool(name="xio", bufs=2) as xio, \
                 tc.tile_pool(name="psc", bufs=1, space="PSUM") as psc_pool, \
                 tc.tile_pool(name="pss", bufs=1, space="PSUM") as pss_pool, \
                 tc.tile_pool(name="pwx", bufs=1, space="PSUM") as pwx_pool, \
                 tc.tile_pool(name="pstr", bufs=1, space="PSUM") as pstr_pool, \
                 tc.tile_pool(name="paT", bufs=1, space="PSUM") as paT_pool:
                for t in range(t_steps):
                    # prefetched x_t slice (bf16, contiguous per partition)
                    xt = xio.tile([128, HC * NL], BF16, tag="xt", name=f"xt_{t}")
                    nc.sync.dma_start(xt[:], ap_xTs[t])

                    strips = [
                        pstr_pool.tile([128, 512], F32, tag=f"strip{r}",
                                       name=f"strip{r}_{t}")
                        for r in range(2)
                    ]

                    # -- x_t @ Wx first: the only PE work independent of h, so
                    # it covers the previous step's cell-update tail and keeps
                    # the PE dense (HAM stays warm).
                    for c in range(HC):
                        for r in range(2):
                            for j in range(4):
                                nc.tensor.matmul(
                                    strips[r][32 * j : 32 * (j + 1), :],
                                    xt[:, 32 * c : 32 * (c + 1)],
                                    Wx_sb[:, c * H4 + j * 1024 + r * 512 : c * H4 + j * 1024 + r * 512 + 512],
                                    start=(c == 0),
                                    stop=False,
                                    skip_group_check=True,
                                    tile_position=(0, 32 * j),
                                )

                    # -- scores: cross-sample products, col-tiled 4-wide (tile
                    # j accumulates h-chunks 2j, 2j+1 into partition rows 32j+).
                    # Tiles j=0,1 only need the r=0 half of uTh, so they run
                    # as soon as the previous step's first half-update lands.
                    psc4 = psc_pool.tile([128, 512], F32, tag="psc4")
                    def psc_half(half):
                        for j in (0, 1) if half == 0 else (2, 3):
                            for cc in range(2):
                                c = 2 * j + cc
                                nc.tensor.matmul(
                                    psc4[32 * j : 32 * (j + 1), :],
                                    uTh[:, 32 * c : 32 * (c + 1)],
                                    Asc_sb[:, c * 512 : (c + 1) * 512],
                                    start=(cc == 0),
                                    stop=(cc == 1),
                                    skip_group_check=True,
                                    tile_position=(0, 32 * j),
                                )
                    def wh_half(half):
                        for c in range(4 * half, 4 * half + 4):
                            for r in range(2):
                                for j in range(4):
                                    nc.tensor.matmul(
                                        strips[r][32 * j : 32 * (j + 1), :],
                                        uTh[:, 32 * c : 32 * (c + 1)],
                                        Wh_sb[:, c * H4 + j * 1024 + r * 512 : c * H4 + j * 1024 + r * 512 + 512],
                                        start=False,
                                        stop=False,
                                        skip_group_check=True,
                                        tile_position=(0, 32 * j),
                                    )
                    psc_half(0)
                    wh_half(0)
                    psc_half(1)
                    wh_half(1)

                    # diag extract + partial reduce (DVE), split by halves so
                    # the first half runs as soon as score tiles 0/1 stop —
                    # only the j=2,3 half sits behind the late uTh update
                    scm = wrk.tile([128, 512], F32, tag="scm")
                    s4 = wrk.tile([128, 16], F32, tag="s4")
                    for hf in range(2):
                        rows = slice(64 * hf, 64 * (hf + 1))
                        nc.vector.tensor_mul(
                            scm[rows, :], psc4[rows, :], Mdiag4[rows, :]
                        )
                        nc.vector.tensor_reduce(
                            s4[rows, :],
                            scm[rows, :].rearrange("p (k n) -> p k n", k=16),
                            axis=AX.X,
                            op=ALU.add,
                        )
                    # sum the 4 col-tile partials with a tiny f32 matmul
                    scores = pss_pool.tile([32, 16], F32, tag="scores")
                    nc.tensor.matmul(scores[:], E32[:], s4[:], start=True, stop=True)

                    # |scores| is O(1) (h in (-1,1), scaled by 1/sqrt(H)):
                    # skip the max-subtract, exp cannot overflow.
                    ex = wrk.tile([32, 16], F32, tag="ex")
                    esum = wrk.tile([32, 1], F32, tag="esum")
                    nc.scalar.activation(
                        ex[:], scores[:], AF.Exp, scale=1.0, accum_out=esum[:]
                    )
                    rcp = wrk.tile([32, 1], F32, tag="rcp")
                    nc.vector.reciprocal(rcp[:], esum[:])
                    # normalize + cast in one op, transpose in bf16
                    nc.vector.tensor_scalar_mul(wsq[:, 0:16], ex[:], rcp[:])
                    wT = wrk.tile([32, 32], BF16, tag="wT")
                    nc.vector.transpose(wT[:], wsq[:])
                    pwx = pwx_pool.tile([128, 32], F32, tag="pwx")
                    nc.tensor.matmul(pwx[:], E16[:], wT[0:16, :], start=True, stop=True)
                    # block-diagonal expansion, split so the first P matmuls
                    # start after half the broadcast multiply
                    masked = wrk.tile([128, 512], BF16, tag="masked")
                    for gh in range(2):
                        nc.vector.tensor_mul(
                            masked[:, 256 * gh : 256 * (gh + 1)].rearrange(
                                "p (b n) -> p b n", b=8
                            ),
                            pwx[:, 0:32].rearrange("p (o n) -> p o n", o=1).broadcast_to(
                                [128, 8, 32]
                            ),
                            M32R[:, 256 * gh : 256 * (gh + 1)].rearrange(
                                "p (b n) -> p b n", b=8
                            ),
                        )

                    # -- attention contribution for both strips first, so the
                    # PSUM->SBUF staging copies overlap with the P matmuls
                    for r in range(2):
                        for g in range(G):
                            for j in range(4):
                                nc.tensor.matmul(
                                    strips[r][32 * j : 32 * (j + 1), :],
                                    masked[:, g * 128 + 32 * j : g * 128 + 32 * j + 32],
                                    P_sb[:, g * H4 + j * 1024 + r * 512 : g * H4 + j * 1024 + r * 512 + 512],
                                    start=False,
                                    stop=(g == G - 1),
                                    skip_group_check=True,
                                    tile_position=(0, 32 * j),
                                )
                    # PSUM -> SBUF staging for the PE transpose: r=0 on ScalarE
                    # (faster PSUM port), r=1 on VectorE, running in parallel
                    pats = []
                    for r in range(2):
                        sg = wrk.tile([128, 512], F32, tag=f"sg{r}")
                        (nc.scalar.copy if r == 0 else nc.vector.tensor_copy)(
                            sg[:], strips[r][:]
                        )
                        pat = paT_pool.tile([128, 512], F32, tag=f"pat{r}")
                        for q in range(4):
                            nc.tensor.matmul(
                                pat[:, 128 * q : 128 * (q + 1)],
                                sg[:, 128 * q : 128 * (q + 1)],
                                eye[:],
                                is_transpose=True,
                                start=(q == 0),
                                stop=(q == 3),
                            )
                        pats.append(pat)

                    for r in range(2):
                        pat = pats[r]
                        # one tanh for all four gates: the host pre-scales the
                        # g-gate weight columns by 2, so tanh(a*0.5) yields
                        # tanh(a_ifo/2) for i/f/o and tanh(a_g) for g. The
                        # sigmoid affine 0.5*(1+t) is folded into the cell
                        # math via scalar_tensor_tensor with cT keeping 2c.
                        act = wrk.tile([128, 512], F32, tag=f"act{r}")
                        nc.scalar.activation(act[:], pat[:], AF.Tanh, scale=0.5)
                        ti_v = q4(act[:])[:, :, 0:32]
                        tf_v = q4(act[:])[:, :, 32:64]
                        to_v = q4(act[:])[:, :, 64:96]
                        g_v = q4(act[:])[:, :, 96:128]
                        cview = cT[:, 128 * r : 128 * (r + 1)].rearrange(
                            "p (q n) -> p q n", q=4
                        )
                        # 2ig = (ti + 1) * g ; 4fc = (tf + 1) * C2
                        ig = wrk.tile([128, 128], F32, tag=f"ig{r}")
                        nc.vector.scalar_tensor_tensor(
                            q4(ig[:]), ti_v, 1.0, g_v, ALU.add, ALU.mult
                        )
                        fc = wrk.tile([128, 128], F32, tag=f"fc{r}")
                        nc.vector.scalar_tensor_tensor(
                            q4(fc[:]), tf_v, 1.0, cview, ALU.add, ALU.mult
                        )
                        # C2' = 2(fc + ig) = 4fc * 0.5 + 2ig
                        nc.vector.scalar_tensor_tensor(
                            cview, q4(fc[:]), 0.5, q4(ig[:]), ALU.mult, ALU.add
                        )
                        tch = wrk.tile([128, 128], F32, tag=f"tch{r}")
                        nc.scalar.activation(
                            tch[:], cT[:, 128 * r : 128 * (r + 1)], AF.Tanh, scale=0.5
                        )
                        # uTh <- 2h = (to + 1) * tanh(c') directly (bf16 cast
                        # in the same op — this is the cross-step critical path)
                        nc.vector.scalar_tensor_tensor(
                            uTh[:, 128 * r : 128 * (r + 1)].rearrange(
                                "p (q n) -> p q n", q=4
                            ),
                            to_v, 1.0,
                            tch[:].rearrange("p (q n) -> p q n", q=4),
                            ALU.add, ALU.mult,
                        )
                        # f32 2h for the output DMA (off the critical path)
                        h2 = wrk.tile([128, 128], F32, tag=f"h2{r}")
                        nc.vector.scalar_tensor_tensor(
                            h2[:].rearrange("p (q n) -> p q n", q=4),
                            to_v, 1.0,
                            tch[:].rearrange("p (q n) -> p q n", q=4),
                            ALU.add, ALU.mult,
                        )
                        h32 = wrk.tile([128, 128], F32, tag=f"h32{r}")
                        nc.gpsimd.tensor_scalar(
                            h32[:], h2[:], 0.5, None, ALU.mult
                        )
                        nc.sync.dma_start(
                            outT[t, 512 * r : 512 * (r + 1), :].rearrange(
                                "(q p) n -> p q n", p=128
                            ),
                            h32[:].rearrange("p (q n) -> p q n", q=4),
                        )
    nc.compile()
    return nc


def _prep_shards(inputs):
    x = np.asarray(inputs["x"], np.float32)
    A = np.asarray(inputs["A"], np.float32)
    Wx = np.asarray(inputs["Wx"], np.float32)
    Wh = np.asarray(inputs["Wh"], np.float32)
    Wattn = np.asarray(inputs["Wattn"], np.float32)
    b = np.asarray(inputs["b"], np.float32)

    # The kernel keeps uTh = 2h (so Wh absorbs a 1/2) and evaluates all four
    # gates with a single tanh(a/2): the g-gate weight columns absorb a 2.
    gscale = np.ones((1, H4), np.float32)
    gscale[0, 3 * H :] = 2.0
    Wx_bf = np.ascontiguousarray((Wx * gscale).astype(BF))
    Wh_bf = np.ascontiguousarray((Wh * 0.5 * gscale).astype(BF))
    Wa_bf = np.ascontiguousarray((Wattn * gscale).astype(BF))
    b_bf = np.ascontiguousarray((b.reshape(1, H4) * gscale).astype(BF))

    in_maps = []
    for i in range(NCORES):
        ns = slice(NL * i, NL * (i + 1))
        # xTs[t, p, c, n] = x[n, t, 128 c + p]
        xTs = x[ns].transpose(1, 2, 0).reshape(T, HC, 128, NL).transpose(0, 2, 1, 3)
        xTs = xTs.reshape(T, 128, HC * NL)
        Asc = A[ns].reshape(NL, H, 16).transpose(1, 2, 0).reshape(H, 512)
        in_maps.append(
            {
                "xTs": np.ascontiguousarray(xTs.astype(BF)),
                "Asc": np.ascontiguousarray(Asc.astype(BF)),
                "Wx": Wx_bf,
                "Wh": Wh_bf,
                "Wattn": Wa_bf,
                "brow": b_bf,
            }
        )
    return in_maps


def _get_nc():
    global _built
    if _built is None:
        _built = _build_nc()
    return _built


def _run(inputs, **kwargs):
    nc = _get_nc()
    in_maps = _prep_shards(inputs)
    res = bass_utils.run_bass_kernel_spmd(
        nc, in_maps, core_ids=list(range(NCORES)), **kwargs
    )
    out = np.empty((N, T, H), np.float32)
    for i in range(NCORES):
        out[NL * i : NL * (i + 1)] = res.results[i]["outT"].transpose(2, 0, 1)
    return out, res


def kernel(**inputs):
    out, _ = _run(inputs)
    return out


# revision 24
# speedup vs baseline: 1.1645x; 1.1645x over previous
"""Trainium2 Bass kernel for nn_CaptioningRNN (attention LSTM over T=64).

Data-parallel over the batch: N=256 samples split across 8 NeuronCores
(32 samples/core), weights replicated, no collectives.

Per-core design (v3 — fully fused step loop):
  - No xproj prepass: x_t @ Wx accumulates into the same PSUM strips as
    h @ Wh and the attention term, so there is no DRAM scratch round trip
    and the TensorEngine stays dense (HAM stays warm).
  - Single ACT table set (exp_and_others): sigmoid is computed as
    0.5*(1+tanh(x/2)) so the per-step Exp (softmax) and Tanh (gates) never
    force an activation-table reload.
  - P precompute: P[n,k,:] = A[n,:,k] @ Wattn + b once; since softmax
    weights sum to 1, folding b into P applies the bias exactly.
  - Per step: scores via cross-sample matmuls + masked diag reduce;
    softmax (no max-subtract — scores are O(1) bounded); w expanded to the
    (k, n_g) block-diagonal stationary via one-hot matmul + broadcast mask
    mul; gates = h@Wh + x_t@Wx + sum_k w_k P_k in 2 column-tiled PSUM
    strips; strips transposed on PE; cell math in h-on-partition space.
  - Output written transposed [t, h, n]; host reassembles to (N, T, H).
"""

import numpy as np
import ml_dtypes

import concourse.bacc as bacc
import concourse.mybir as mybir
from concourse import bass_utils
from concourse.tile import TileContext

F32, BF16 = mybir.dt.float32, mybir.dt.bfloat16
AF = mybir.ActivationFunctionType
ALU = mybir.AluOpType
AX = mybir.AxisListType
BF = ml_dtypes.bfloat16

N, T, D, H = 256, 64, 1024, 1024
NCORES = 8
NL = N // NCORES          # 32 samples per core
HC = 8                    # 128-row chunks of D/H
G, GS = 4, 8              # sample groups of 8 (for the (k, n_g) 128-partition layout)
H4 = 4 * H                # 4096 gate columns

_built = None


def _consts():
    # E16[k', k] one-hot: expands wT rows onto the 128-partition (k, n_g) axis.
    e16 = np.zeros((16, 128), dtype=BF)
    for k in range(16):
        e16[k, 8 * k : 8 * k + 8] = 1
    # M32R[p, 128 g + 32 rep + m] = (m % 8 == p % 8) & (m // 8 == g):
    # block-diagonal mask producing masked = w[m, k(p)] only for group-g
    # samples, replicated over the 4 column-tile strips.
    p = np.arange(128)[:, None]
    m = np.arange(32)[None, :]
    m32r = np.zeros((128, 512), dtype=BF)
    for g in range(4):
        blk = ((m % 8 == p % 8) & (m // 8 == g)).astype(BF)
        for rep in range(4):
            m32r[:, 128 * g + 32 * rep : 128 * g + 32 * rep + 32] = blk
    # Mdiag4[32 j + n, 32 k + n'] = (n == n') / 32: extracts the diagonal of
    # the col-tiled cross-sample score partials and applies the 1/sqrt(H)
    # softmax scale (same pattern for each of the 4 partition tiles).
    md = np.zeros((32, 512), dtype=np.float32)
    n_ = np.arange(32)
    for k in range(16):
        md[n_, 32 * k + n_] = 1.0 / 32.0
    # uTh holds 2h, so the score scale absorbs an extra 1/2 (1/64 total)
    md4 = np.tile(md, (4, 1)) * 0.5
    # E32[p, m] = (p % 32 == m): sums the 4 col-tiled score partials (f32
    # stationary so the tiny N=16 matmul needs no cast of its moving operand).
    e32 = np.zeros((128, 32), dtype=np.float32)
    e32[np.arange(128), np.arange(128) % 32] = 1
    # row-0 selector for the rank-1 bias accumulation into P.
    er0 = np.zeros((128, 128), dtype=BF)
    er0[0, :] = 1
    return e16, m32r, md4, e32, er0


def _build_nc(t_steps=T):
    nc = bacc.Bacc(trn_type="TRN2", target_bir_lowering=False, debug=False)

    # xTs[t, p, c, n] = x[n, t, 128 c + p] (bf16) — one contiguous DMA/step
    ap_xTs = nc.dram_tensor("xTs", [T, 128, HC * NL], BF16, kind="ExternalInput").ap()
    ap_Asc = nc.dram_tensor("Asc", [H, 512], BF16, kind="ExternalInput").ap()
    ap_Wx = nc.dram_tensor("Wx", [D, H4], BF16, kind="ExternalInput").ap()
    ap_Wh = nc.dram_tensor("Wh", [H, H4], BF16, kind="ExternalInput").ap()
    ap_Wattn = nc.dram_tensor("Wattn", [H, H4], BF16, kind="ExternalInput").ap()
    ap_brow = nc.dram_tensor("brow", [1, H4], BF16, kind="ExternalInput").ap()
    outT = nc.dram_tensor("outT", [T, H, NL], BF16, kind="ExternalOutput").ap()

    e16_np, m32r_np, md4_np, e32_np, er0_np = _consts()
    eye_d = nc.inline_tensor(np.eye(128, dtype=np.float32), "c_eye")
    e16_d = nc.inline_tensor(e16_np, "c_e16")
    m32r_d = nc.inline_tensor(m32r_np, "c_m32r")
    md4_d = nc.inline_tensor(md4_np, "c_mdiag4")
    e32_d = nc.inline_tensor(e32_np, "c_e32")
    er0_d = nc.inline_tensor(er0_np, "c_er0")

    q4 = lambda ap: ap.rearrange("p (q c) -> p q c", q=4)

    with TileContext(nc) as tc:
        with tc.tile_pool(name="pers", bufs=1) as pers:
            Wh_sb = pers.tile([128, HC * H4], BF16, tag="Wh")
            Wx_sb = pers.tile([128, HC * H4], BF16, tag="Wx")
            Asc_sb = pers.tile([128, HC * 512], BF16, tag="Asc")
            P_sb = pers.tile([128, G * H4], BF16, tag="P")
            uTh = pers.tile([128, HC * NL], BF16, tag="uTh")
            cT = pers.tile([128, 256], F32, tag="cT")
            eye = pers.tile([128, 128], F32, tag="eye")
            E16 = pers.tile([16, 128], BF16, tag="E16")
            M32R = pers.tile([128, 512], BF16, tag="M32R")
            Mdiag4 = pers.tile([128, 512], F32, tag="Mdiag4")
            E32 = pers.tile([128, 32], F32, tag="E32")
            ER0 = pers.tile([128, 128], BF16, tag="ER0")
            wsq = pers.tile([32, 32], BF16, tag="wsq")

            # Asc + the first Wattn blocks are what phase B needs — issue
            # those DMAs before the big Wh/Wx loads so the P matmuls can
            # start early (the Wh/Wx weights are only needed at step 0).
            for c in range(HC):
                nc.sync.dma_start(
                    Asc_sb[:, c * 512 : (c + 1) * 512],
                    ap_Asc[128 * c : 128 * (c + 1), :],
                )
            nc.sync.dma_start(eye[:], eye_d.ap()[:])
            nc.sync.dma_start(E16[:], e16_d.ap()[:])
            nc.sync.dma_start(M32R[:], m32r_d.ap()[:])
            nc.sync.dma_start(Mdiag4[:], md4_d.ap()[:])
            nc.sync.dma_start(E32[:], e32_d.ap()[:])
            nc.sync.dma_start(ER0[:], er0_d.ap()[:])
            nc.gpsimd.memset(wsq[:], 0.0)

            # ------------- phase B: P precompute (+bias) + h0/c0 init -------------
            with tc.tile_pool(name="php1", bufs=1) as php1, \
                 tc.tile_pool(name="php", bufs=2) as php, \
                 tc.tile_pool(name="psP", bufs=2, space="PSUM") as psP:
                for c in range(HC):
                    h0s = php.tile([128, 32], F32, tag="h0s")
                    nc.vector.tensor_reduce(
                        h0s[:],
                        Asc_sb[:, c * 512 : (c + 1) * 512].rearrange(
                            "p (k n) -> p n k", k=16
                        ),
                        axis=AX.X,
                        op=ALU.add,
                    )
                    # cT holds C2 = 2*c and uTh holds 2*h throughout (the cell
                    # update keeps the doubled scale; tanh reads with scale=0.5
                    # and the host pre-scales Wh by 1/2)
                    nc.vector.tensor_scalar_mul(
                        cT[:, 32 * c : 32 * (c + 1)], h0s[:], 1.0 / 8.0
                    )
                    nc.vector.tensor_scalar_mul(
                        uTh[:, 32 * c : 32 * (c + 1)], h0s[:], 1.0 / 8.0
                    )
                # contiguous staging of the group-selected A columns so the
                # matmul stationary operand has a single free dim
                Ag = php1.tile([128, G * HC * 128], BF16, tag="Ag")
                for g in range(G):
                    for c in range(HC):
                        nc.vector.tensor_copy(
                            Ag[:, (g * HC + c) * 128 : (g * HC + c) * 128 + 128],
                            Asc_sb[:, c * 512 : (c + 1) * 512].rearrange(
                                "p (k n) -> p k n", k=16
                            )[:, :, GS * g : GS * (g + 1)],
                        )
                bmv = php1.tile([128, 512], BF16, tag="bmv")
                nc.gpsimd.memset(bmv[:], 0.0)
                for blk in range(8):
                    Wab = php.tile([128, HC * 512], BF16, tag="Wab")
                    for c in range(HC):
                        nc.sync.dma_start(
                            Wab[:, c * 512 : (c + 1) * 512],
                            ap_Wattn[128 * c : 128 * (c + 1), 512 * blk : 512 * (blk + 1)],
                        )
                    if blk == 1:
                        # big Wh/Wx loads issued after the first Wattn blocks:
                        # they are only needed once the recurrence starts
                        for c in range(HC):
                            nc.sync.dma_start(
                                Wh_sb[:, c * H4 : (c + 1) * H4],
                                ap_Wh[128 * c : 128 * (c + 1), :],
                            )
                            nc.sync.dma_start(
                                Wx_sb[:, c * H4 : (c + 1) * H4],
                                ap_Wx[128 * c : 128 * (c + 1), :],
                            )
                    bsl = php.tile([1, 512], BF16, tag="bsl")
                    nc.sync.dma_start(bsl[:], ap_brow[0:1, 512 * blk : 512 * (blk + 1)])
                    nc.vector.tensor_copy(bmv[0:1, :], bsl[:])
                    for g in range(G):
                        psp = psP.tile([128, 512], F32, tag="psp")
                        for c in range(HC):
                            nc.tensor.matmul(
                                psp[:],
                                Ag[:, (g * HC + c) * 128 : (g * HC + c) * 128 + 128],
                                Wab[:, c * 512 : (c + 1) * 512],
                                start=(c == 0),
                                stop=False,
                            )
                        # rank-1 accumulation: adds b[blk cols] to every row
                        # (softmax weights sum to 1, so this applies +b exactly)
                        nc.tensor.matmul(
                            psp[:], ER0[:], bmv[:], start=False, stop=True
                        )
                        nc.vector.tensor_copy(
                            P_sb[:, g * H4 + 512 * blk : g * H4 + 512 * (blk + 1)],
                            psp[:],
                        )

            # ---------------------- phase C: recurrence ----------------------
            with tc.tile_pool(name="wrk", bufs=2) as wrk, \
                 tc.tile_pool(name="xio", bufs=2) as xio, \
                 tc.tile_pool(name="psc", bufs=1, space="PSUM") as psc_pool, \
                 tc.tile_pool(name="pss", bufs=1, space="PSUM") as pss_pool, \
                 tc.tile_pool(name="pwx", bufs=1, space="PSUM") as pwx_pool, \
                 tc.tile_pool(name="pstr", bufs=1, space="PSUM") as pstr_pool, \
                 tc.tile_pool(name="paT", bufs=1, space="PSUM") as paT_pool:
                for t in range(t_steps):
                    # prefetched x_t slice (bf16, contiguous per partition)
                    xt = xio.tile([128, HC * NL], BF16, tag="xt", name=f"xt_{t}")
                    nc.sync.dma_start(xt[:], ap_xTs[t])

                    strips = [
                        pstr_pool.tile([128, 512], F32, tag=f"strip{r}",
                                       name=f"strip{r}_{t}")
                        for r in range(2)
                    ]

                    # -- x_t @ Wx first: the only PE work independent of h, so
                    # it covers the previous step's cell-update tail and keeps
                    # the PE dense (HAM stays warm).
                    for c in range(HC):
                        for r in range(2):
                            for j in range(4):
                                nc.tensor.matmul(
                                    strips[r][32 * j : 32 * (j + 1), :],
                                    xt[:, 32 * c : 32 * (c + 1)],
                                    Wx_sb[:, c * H4 + j * 1024 + r * 512 : c * H4 + j * 1024 + r * 512 + 512],
                                    start=(c == 0),
                                    stop=False,
                                    skip_group_check=True,
                                    tile_position=(0, 32 * j),
                                )

                    # -- scores: cross-sample products, col-tiled 4-wide (tile
                    # j accumulates h-chunks 2j, 2j+1 into partition rows 32j+).
                    # Tiles j=0,1 only need the r=0 half of uTh, so they run
                    # as soon as the previous step's first half-update lands.
                    psc4 = psc_pool.tile([128, 512], F32, tag="psc4")
                    def psc_half(half):
                        for j in (0, 1) if half == 0 else (2, 3):
                            for cc in range(2):
                                c = 2 * j + cc
                                nc.tensor.matmul(
                                    psc4[32 * j : 32 * (j + 1), :],
                                    uTh[:, 32 * c : 32 * (c + 1)],
                                    Asc_sb[:, c * 512 : (c + 1) * 512],
                                    start=(cc == 0),
                                    stop=(cc == 1),
                                    skip_group_check=True,
                                    tile_position=(0, 32 * j),
                                )
                    def wh_half(half):
                        for c in range(4 * half, 4 * half + 4):
                            for r in range(2):
                                for j in range(4):
                                    nc.tensor.matmul(
                                        strips[r][32 * j : 32 * (j + 1), :],
                                        uTh[:, 32 * c : 32 * (c + 1)],
                                        Wh_sb[:, c * H4 + j * 1024 + r * 512 : c * H4 + j * 1024 + r * 512 + 512],
                                        start=False,
                                        stop=False,
                                        skip_group_check=True,
                                        tile_position=(0, 32 * j),
                                    )
                    psc_half(0)
                    wh_half(0)
                    psc_half(1)
                    wh_half(1)

                    # diag extract + partial reduce (DVE; cost scales with the
                    # free dim, so one full-width op each — never row-split)
                    scm = wrk.tile([128, 512], F32, tag="scm")
                    s4 = wrk.tile([128, 16], F32, tag="s4")
                    nc.vector.tensor_mul(scm[:], psc4[:], Mdiag4[:])
                    nc.vector.tensor_reduce(
                        s4[:],
                        scm[:].rearrange("p (k n) -> p k n", k=16),
                        axis=AX.X,
                        op=ALU.add,
                    )
                    # sum the 4 col-tile partials with a tiny f32 matmul
                    scores = pss_pool.tile([32, 16], F32, tag="scores")
                    nc.tensor.matmul(scores[:], E32[:], s4[:], start=True, stop=True)

                    # |scores| is O(1) (h in (-1,1), scaled by 1/sqrt(H)):
                    # skip the max-subtract, exp cannot overflow.
                    ex = wrk.tile([32, 16], F32, tag="ex")
                    esum = wrk.tile([32, 1], F32, tag="esum")
                    nc.scalar.activation(
                        ex[:], scores[:], AF.Exp, scale=1.0, accum_out=esum[:]
                    )
                    rcp = wrk.tile([32, 1], F32, tag="rcp")
                    nc.vector.reciprocal(rcp[:], esum[:])
                    # normalize + cast in one op, transpose in bf16
                    nc.vector.tensor_scalar_mul(wsq[:, 0:16], ex[:], rcp[:])
                    wT = wrk.tile([32, 32], BF16, tag="wT")
                    nc.vector.transpose(wT[:], wsq[:])
                    pwx = pwx_pool.tile([128, 32], F32, tag="pwx")
                    nc.tensor.matmul(pwx[:], E16[:], wT[0:16, :], start=True, stop=True)
                    # block-diagonal expansion, split so the first P matmuls
                    # start after half the broadcast multiply
                    masked = wrk.tile([128, 512], BF16, tag="masked")
                    for gh in range(2):
                        nc.vector.tensor_mul(
                            masked[:, 256 * gh : 256 * (gh + 1)].rearrange(
                                "p (b n) -> p b n", b=8
                            ),
                            pwx[:, 0:32].rearrange("p (o n) -> p o n", o=1).broadcast_to(
                                [128, 8, 32]
                            ),
                            M32R[:, 256 * gh : 256 * (gh + 1)].rearrange(
                                "p (b n) -> p b n", b=8
                            ),
                        )

                    # -- attention contribution for both strips first, so the
                    # PSUM->SBUF staging copies overlap with the P matmuls
                    for r in range(2):
                        for g in range(G):
                            for j in range(4):
                                nc.tensor.matmul(
                                    strips[r][32 * j : 32 * (j + 1), :],
                                    masked[:, g * 128 + 32 * j : g * 128 + 32 * j + 32],
                                    P_sb[:, g * H4 + j * 1024 + r * 512 : g * H4 + j * 1024 + r * 512 + 512],
                                    start=False,
                                    stop=(g == G - 1),
                                    skip_group_check=True,
                                    tile_position=(0, 32 * j),
                                )
                    # PSUM -> SBUF staging for the PE transpose: r=0 on ScalarE
                    # (faster PSUM port), r=1 on VectorE, running in parallel
                    pats = []
                    for r in range(2):
                        sg = wrk.tile([128, 512], F32, tag=f"sg{r}")
                        (nc.scalar.copy if r == 0 else nc.vector.tensor_copy)(
                            sg[:], strips[r][:]
                        )
                        pat = paT_pool.tile([128, 512], F32, tag=f"pat{r}")
                        for q in range(4):
                            nc.tensor.matmul(
                                pat[:, 128 * q : 128 * (q + 1)],
                                sg[:, 128 * q : 128 * (q + 1)],
                                eye[:],
                                is_transpose=True,
                                start=(q == 0),
                                stop=(q == 3),
                            )
                        pats.append(pat)

                    for r in range(2):
                        pat = pats[r]
                        # one tanh for all four gates: the host pre-scales the
                        # g-gate weight columns by 2, so tanh(a*0.5) yields
                        # tanh(a_ifo/2) for i/f/o and tanh(a_g) for g. The
                        # sigmoid affine 0.5*(1+t) is folded into the cell
                        # math via scalar_tensor_tensor with cT keeping 2c.
                        act = wrk.tile([128, 512], F32, tag=f"act{r}")
                        nc.scalar.activation(act[:], pat[:], AF.Tanh, scale=0.5)
                        ti_v = q4(act[:])[:, :, 0:32]
                        tf_v = q4(act[:])[:, :, 32:64]
                        to_v = q4(act[:])[:, :, 64:96]
                        g_v = q4(act[:])[:, :, 96:128]
                        cview = cT[:, 128 * r : 128 * (r + 1)].rearrange(
                            "p (q n) -> p q n", q=4
                        )
                        # 2ig = (ti + 1) * g ; 4fc = (tf + 1) * C2
                        ig = wrk.tile([128, 128], F32, tag=f"ig{r}")
                        nc.vector.scalar_tensor_tensor(
                            q4(ig[:]), ti_v, 1.0, g_v, ALU.add, ALU.mult
                        )
                        fc = wrk.tile([128, 128], F32, tag=f"fc{r}")
                        nc.vector.scalar_tensor_tensor(
                            q4(fc[:]), tf_v, 1.0, cview, ALU.add, ALU.mult
                        )
                        # C2' = 2(fc + ig) = 4fc * 0.5 + 2ig
                        nc.vector.scalar_tensor_tensor(
                            cview, q4(fc[:]), 0.5, q4(ig[:]), ALU.mult, ALU.add
                        )
                        tch = wrk.tile([128, 128], F32, tag=f"tch{r}")
                        nc.scalar.activation(
                            tch[:], cT[:, 128 * r : 128 * (r + 1)], AF.Tanh, scale=0.5
                        )
                        # uTh <- 2h = (to + 1) * tanh(c') directly (bf16 cast
                        # in the same op — this is the cross-step critical path)
                        nc.vector.scalar_tensor_tensor(
                            uTh[:, 128 * r : 128 * (r + 1)].rearrange(
                                "p (q n) -> p q n", q=4
                            ),
                            to_v, 1.0,
                            tch[:].rearrange("p (q n) -> p q n", q=4),
                            ALU.add, ALU.mult,
                        )
                        # output streamed as bf16 2h straight from uTh; the
                        # host casts to f32 and applies the 1/2
                        nc.sync.dma_start(
                            outT[t, 512 * r : 512 * (r + 1), :].rearrange(
                                "(q p) n -> p q n", p=128
                            ),
                            uTh[:, 128 * r : 128 * (r + 1)].rearrange(
                                "p (q n) -> p q n", q=4
                            ),
                        )
    nc.compile()
    return nc


def _prep_shards(inputs):
    x = np.asarray(inputs["x"], np.float32)
    A = np.asarray(inputs["A"], np.float32)
    Wx = np.asarray(inputs["Wx"], np.float32)
    Wh = np.asarray(inputs["Wh"], np.float32)
    Wattn = np.asarray(inputs["Wattn"], np.float32)
    b = np.asarray(inputs["b"], np.float32)

    # The kernel keeps uTh = 2h (so Wh absorbs a 1/2) and evaluates all four
    # gates with a single tanh(a/2): the g-gate weight columns absorb a 2.
    gscale = np.ones((1, H4), np.float32)
    gscale[0, 3 * H :] = 2.0
    Wx_bf = np.ascontiguousarray((Wx * gscale).astype(BF))
    Wh_bf = np.ascontiguousarray((Wh * 0.5 * gscale).astype(BF))
    Wa_bf = np.ascontiguousarray((Wattn * gscale).astype(BF))
    b_bf = np.ascontiguousarray((b.reshape(1, H4) * gscale).astype(BF))

    in_maps = []
    for i in range(NCORES):
        ns = slice(NL * i, NL * (i + 1))
        # xTs[t, p, c, n] = x[n, t, 128 c + p]
        xTs = x[ns].transpose(1, 2, 0).reshape(T, HC, 128, NL).transpose(0, 2, 1, 3)
        xTs = xTs.reshape(T, 128, HC * NL)
        Asc = A[ns].reshape(NL, H, 16).transpose(1, 2, 0).reshape(H, 512)
        in_maps.append(
            {
                "xTs": np.ascontiguousarray(xTs.astype(BF)),
                "Asc": np.ascontiguousarray(Asc.astype(BF)),
                "Wx": Wx_bf,
                "Wh": Wh_bf,
                "Wattn": Wa_bf,
                "brow": b_bf,
            }
        )
    return in_maps


def _get_nc():
    global _built
    if _built is None:
        _built = _build_nc()
    return _built


def _run(inputs, **kwargs):
    nc = _get_nc()
    in_maps = _prep_shards(inputs)
    res = bass_utils.run_bass_kernel_spmd(
        nc, in_maps, core_ids=list(range(NCORES)), **kwargs
    )
    out = np.empty((N, T, H), np.float32)
    for i in range(NCORES):
        # outT holds 2h in bf16 — cast up and halve on the host
        o = res.results[i]["outT"].astype(np.float32) * 0.5
        out[NL * i : NL * (i + 1)] = o.transpose(2, 0, 1)
    return out, res


def kernel(**inputs):
    out, _ = _run(inputs)
    return out


# revision 28
# speedup vs baseline: 1.2525x; 1.0756x over previous
"""Trainium2 Bass kernel for nn_CaptioningRNN (attention LSTM over T=64).

Data-parallel over the batch: N=256 samples split across 8 NeuronCores
(32 samples/core), weights replicated, no collectives.

Per-core design (v3 — fully fused step loop):
  - No xproj prepass: x_t @ Wx accumulates into the same PSUM strips as
    h @ Wh and the attention term, so there is no DRAM scratch round trip
    and the TensorEngine stays dense (HAM stays warm).
  - Single ACT table set (exp_and_others): sigmoid is computed as
    0.5*(1+tanh(x/2)) so the per-step Exp (softmax) and Tanh (gates) never
    force an activation-table reload.
  - P precompute: P[n,k,:] = A[n,:,k] @ Wattn + b once; since softmax
    weights sum to 1, folding b into P applies the bias exactly.
  - Per step: scores via cross-sample matmuls + masked diag reduce;
    softmax (no max-subtract — scores are O(1) bounded); w expanded to the
    (k, n_g) block-diagonal stationary via one-hot matmul + broadcast mask
    mul; gates = h@Wh + x_t@Wx + sum_k w_k P_k in 2 column-tiled PSUM
    strips; strips transposed on PE; cell math in h-on-partition space.
  - Output written transposed [t, h, n]; host reassembles to (N, T, H).
"""

import numpy as np
import ml_dtypes

import concourse.bacc as bacc
import concourse.mybir as mybir
from concourse import bass_utils
from concourse.tile import TileContext

F32, BF16 = mybir.dt.float32, mybir.dt.bfloat16
AF = mybir.ActivationFunctionType
ALU = mybir.AluOpType
AX = mybir.AxisListType
BF = ml_dtypes.bfloat16

N, T, D, H = 256, 64, 1024, 1024
NCORES = 8
NL = N // NCORES          # 32 samples per core
HC = 8                    # 128-row chunks of D/H
G, GS = 4, 8              # sample groups of 8 (for the (k, n_g) 128-partition layout)
H4 = 4 * H                # 4096 gate columns

_built = None


def _consts():
    # E16[k', k] one-hot: expands wT rows onto the 128-partition (k, n_g) axis.
    e16 = np.zeros((16, 128), dtype=BF)
    for k in range(16):
        e16[k, 8 * k : 8 * k + 8] = 1
    # M32R[p, 128 g + 32 rep + m] = (m % 8 == p % 8) & (m // 8 == g):
    # block-diagonal mask producing masked = w[m, k(p)] only for group-g
    # samples, replicated over the 4 column-tile strips.
    p = np.arange(128)[:, None]
    m = np.arange(32)[None, :]
    m32r = np.zeros((128, 512), dtype=BF)
    for g in range(4):
        blk = ((m % 8 == p % 8) & (m // 8 == g)).astype(BF)
        for rep in range(4):
            m32r[:, 128 * g + 32 * rep : 128 * g + 32 * rep + 32] = blk
    # Mdiag4[32 j + n, 32 k + n'] = (n == n') / 32: extracts the diagonal of
    # the col-tiled cross-sample score partials and applies the 1/sqrt(H)
    # softmax scale (same pattern for each of the 4 partition tiles).
    md = np.zeros((32, 512), dtype=np.float32)
    n_ = np.arange(32)
    for k in range(16):
        md[n_, 32 * k + n_] = 1.0 / 32.0
    # uTh holds 2h, so the score scale absorbs an extra 1/2 (1/64 total)
    md4 = np.tile(md, (4, 1)) * 0.5
    # E32[p, m] = (p % 32 == m): sums the 4 col-tiled score partials (f32
    # stationary so the tiny N=16 matmul needs no cast of its moving operand).
    e32 = np.zeros((128, 32), dtype=np.float32)
    e32[np.arange(128), np.arange(128) % 32] = 1
    # row-0 selector for the rank-1 bias accumulation into P.
    er0 = np.zeros((128, 128), dtype=BF)
    er0[0, :] = 1
    return e16, m32r, md4, e32, er0


def _build_nc(t_steps=T):
    nc = bacc.Bacc(trn_type="TRN2", target_bir_lowering=False, debug=False)

    # xTs[t, p, c, n] = x[n, t, 128 c + p] (bf16) — one contiguous DMA/step
    ap_xTs = nc.dram_tensor("xTs", [T, 128, HC * NL], BF16, kind="ExternalInput").ap()
    ap_Asc = nc.dram_tensor("Asc", [H, 512], BF16, kind="ExternalInput").ap()
    ap_Wx = nc.dram_tensor("Wx", [D, H4], BF16, kind="ExternalInput").ap()
    ap_Wh = nc.dram_tensor("Wh", [H, H4], BF16, kind="ExternalInput").ap()
    ap_Wattn = nc.dram_tensor("Wattn", [H, H4], BF16, kind="ExternalInput").ap()
    ap_brow = nc.dram_tensor("brow", [1, H4], BF16, kind="ExternalInput").ap()
    outT = nc.dram_tensor("outT", [T, H, NL], BF16, kind="ExternalOutput").ap()

    e16_np, m32r_np, md4_np, e32_np, er0_np = _consts()
    eye_d = nc.inline_tensor(np.eye(128, dtype=np.float32), "c_eye")
    e16_d = nc.inline_tensor(e16_np, "c_e16")
    m32r_d = nc.inline_tensor(m32r_np, "c_m32r")
    md4_d = nc.inline_tensor(md4_np, "c_mdiag4")
    e32_d = nc.inline_tensor(e32_np, "c_e32")
    er0_d = nc.inline_tensor(er0_np, "c_er0")

    q4 = lambda ap: ap.rearrange("p (q c) -> p q c", q=4)

    with TileContext(nc) as tc:
        with tc.tile_pool(name="pers", bufs=1) as pers:
            Wh_sb = pers.tile([128, HC * H4], BF16, tag="Wh")
            Wx_sb = pers.tile([128, HC * H4], BF16, tag="Wx")
            Asc_sb = pers.tile([128, HC * 512], BF16, tag="Asc")
            P_sb = pers.tile([128, G * H4], BF16, tag="P")
            uTh = pers.tile([128, HC * NL], BF16, tag="uTh")
            cT = pers.tile([128, 256], F32, tag="cT")
            eye = pers.tile([128, 128], F32, tag="eye")
            E16 = pers.tile([16, 128], BF16, tag="E16")
            M32R = pers.tile([128, 512], BF16, tag="M32R")
            Mdiag4 = pers.tile([128, 512], F32, tag="Mdiag4")
            E32 = pers.tile([128, 32], F32, tag="E32")
            ER0 = pers.tile([128, 128], BF16, tag="ER0")
            wsq = pers.tile([32, 32], BF16, tag="wsq")

            # Asc + the first Wattn blocks are what phase B needs — issue
            # those DMAs before the big Wh/Wx loads so the P matmuls can
            # start early (the Wh/Wx weights are only needed at step 0).
            for c in range(HC):
                nc.sync.dma_start(
                    Asc_sb[:, c * 512 : (c + 1) * 512],
                    ap_Asc[128 * c : 128 * (c + 1), :],
                )
            nc.sync.dma_start(eye[:], eye_d.ap()[:])
            nc.sync.dma_start(E16[:], e16_d.ap()[:])
            nc.sync.dma_start(M32R[:], m32r_d.ap()[:])
            nc.sync.dma_start(Mdiag4[:], md4_d.ap()[:])
            nc.sync.dma_start(E32[:], e32_d.ap()[:])
            nc.sync.dma_start(ER0[:], er0_d.ap()[:])
            nc.gpsimd.memset(wsq[:], 0.0)

            # ------------- phase B: P precompute (+bias) + h0/c0 init -------------
            with tc.tile_pool(name="php1", bufs=1) as php1, \
                 tc.tile_pool(name="php", bufs=2) as php, \
                 tc.tile_pool(name="psP", bufs=2, space="PSUM") as psP:
                for c in range(HC):
                    h0s = php.tile([128, 32], F32, tag="h0s")
                    nc.vector.tensor_reduce(
                        h0s[:],
                        Asc_sb[:, c * 512 : (c + 1) * 512].rearrange(
                            "p (k n) -> p n k", k=16
                        ),
                        axis=AX.X,
                        op=ALU.add,
                    )
                    # cT holds C2 = 2*c and uTh holds 2*h throughout (the cell
                    # update keeps the doubled scale; tanh reads with scale=0.5
                    # and the host pre-scales Wh by 1/2)
                    nc.vector.tensor_scalar_mul(
                        cT[:, 32 * c : 32 * (c + 1)], h0s[:], 1.0 / 8.0
                    )
                    nc.vector.tensor_scalar_mul(
                        uTh[:, 32 * c : 32 * (c + 1)], h0s[:], 1.0 / 8.0
                    )
                # contiguous staging of the group-selected A columns so the
                # matmul stationary operand has a single free dim
                Ag = php1.tile([128, G * HC * 128], BF16, tag="Ag")
                for g in range(G):
                    for c in range(HC):
                        nc.vector.tensor_copy(
                            Ag[:, (g * HC + c) * 128 : (g * HC + c) * 128 + 128],
                            Asc_sb[:, c * 512 : (c + 1) * 512].rearrange(
                                "p (k n) -> p k n", k=16
                            )[:, :, GS * g : GS * (g + 1)],
                        )
                bmv = php1.tile([128, 512], BF16, tag="bmv")
                nc.gpsimd.memset(bmv[:], 0.0)
                for blk in range(8):
                    Wab = php.tile([128, HC * 512], BF16, tag="Wab")
                    for c in range(HC):
                        nc.sync.dma_start(
                            Wab[:, c * 512 : (c + 1) * 512],
                            ap_Wattn[128 * c : 128 * (c + 1), 512 * blk : 512 * (blk + 1)],
                        )
                    if blk == 1:
                        # big Wh/Wx loads issued after the first Wattn blocks:
                        # they are only needed once the recurrence starts
                        for c in range(HC):
                            nc.sync.dma_start(
                                Wh_sb[:, c * H4 : (c + 1) * H4],
                                ap_Wh[128 * c : 128 * (c + 1), :],
                            )
                            nc.sync.dma_start(
                                Wx_sb[:, c * H4 : (c + 1) * H4],
                                ap_Wx[128 * c : 128 * (c + 1), :],
                            )
                    bsl = php.tile([1, 512], BF16, tag="bsl")
                    nc.sync.dma_start(bsl[:], ap_brow[0:1, 512 * blk : 512 * (blk + 1)])
                    nc.vector.tensor_copy(bmv[0:1, :], bsl[:])
                    for g in range(G):
                        psp = psP.tile([128, 512], F32, tag="psp")
                        for c in range(HC):
                            nc.tensor.matmul(
                                psp[:],
                                Ag[:, (g * HC + c) * 128 : (g * HC + c) * 128 + 128],
                                Wab[:, c * 512 : (c + 1) * 512],
                                start=(c == 0),
                                stop=False,
                            )
                        # rank-1 accumulation: adds b[blk cols] to every row
                        # (softmax weights sum to 1, so this applies +b exactly)
                        nc.tensor.matmul(
                            psp[:], ER0[:], bmv[:], start=False, stop=True
                        )
                        nc.vector.tensor_copy(
                            P_sb[:, g * H4 + 512 * blk : g * H4 + 512 * (blk + 1)],
                            psp[:],
                        )

            # ---------------------- phase C: recurrence ----------------------
            with tc.tile_pool(name="wrk", bufs=2) as wrk, \
                 tc.tile_pool(name="xio", bufs=2) as xio, \
                 tc.tile_pool(name="psc", bufs=1, space="PSUM") as psc_pool, \
                 tc.tile_pool(name="pss", bufs=1, space="PSUM") as pss_pool, \
                 tc.tile_pool(name="pstr", bufs=2, space="PSUM") as pstr_pool, \
                 tc.tile_pool(name="paT", bufs=1, space="PSUM") as paT_pool:
                for t in range(t_steps):
                    # prefetched x_t slice (bf16, contiguous per partition)
                    xt = xio.tile([128, HC * NL], BF16, tag="xt", name=f"xt_{t}")
                    nc.sync.dma_start(xt[:], ap_xTs[t])

                    strips = [
                        pstr_pool.tile([128, 512], F32, tag=f"strip{r}",
                                       name=f"strip{r}_{t}")
                        for r in range(2)
                    ]

                    # -- x_t @ Wx first: the only PE work independent of h, so
                    # it covers the previous step's cell-update tail and keeps
                    # the PE dense (HAM stays warm).
                    for c in range(HC):
                        for r in range(2):
                            for j in range(4):
                                nc.tensor.matmul(
                                    strips[r][32 * j : 32 * (j + 1), :],
                                    xt[:, 32 * c : 32 * (c + 1)],
                                    Wx_sb[:, c * H4 + j * 1024 + r * 512 : c * H4 + j * 1024 + r * 512 + 512],
                                    start=(c == 0),
                                    stop=False,
                                    skip_group_check=True,
                                    tile_position=(0, 32 * j),
                                )

                    # -- scores: cross-sample products, col-tiled 4-wide (tile
                    # j accumulates h-chunks 2j, 2j+1 into partition rows 32j+).
                    # Tiles j=0,1 only need the r=0 half of uTh, so they run
                    # as soon as the previous step's first half-update lands.
                    psc4 = psc_pool.tile([128, 512], F32, tag="psc4")
                    def psc_half(half):
                        for j in (0, 1) if half == 0 else (2, 3):
                            for cc in range(2):
                                c = 2 * j + cc
                                nc.tensor.matmul(
                                    psc4[32 * j : 32 * (j + 1), :],
                                    uTh[:, 32 * c : 32 * (c + 1)],
                                    Asc_sb[:, c * 512 : (c + 1) * 512],
                                    start=(cc == 0),
                                    stop=(cc == 1),
                                    skip_group_check=True,
                                    tile_position=(0, 32 * j),
                                )
                    def wh_half(half):
                        for c in range(4 * half, 4 * half + 4):
                            for r in range(2):
                                for j in range(4):
                                    nc.tensor.matmul(
                                        strips[r][32 * j : 32 * (j + 1), :],
                                        uTh[:, 32 * c : 32 * (c + 1)],
                                        Wh_sb[:, c * H4 + j * 1024 + r * 512 : c * H4 + j * 1024 + r * 512 + 512],
                                        start=False,
                                        stop=False,
                                        skip_group_check=True,
                                        tile_position=(0, 32 * j),
                                    )
                    psc_half(0)
                    wh_half(0)
                    psc_half(1)
                    wh_half(1)

                    # diag extract + partial reduce (DVE; cost scales with the
                    # free dim, so one full-width op each — never row-split)
                    scm = wrk.tile([128, 512], F32, tag="scm")
                    s4 = wrk.tile([128, 16], F32, tag="s4")
                    nc.vector.tensor_mul(scm[:], psc4[:], Mdiag4[:])
                    nc.vector.tensor_reduce(
                        s4[:],
                        scm[:].rearrange("p (k n) -> p k n", k=16),
                        axis=AX.X,
                        op=ALU.add,
                    )
                    # sum the 4 col-tile partials with a tiny f32 matmul.
                    # scores and pwx share one PSUM bank (both tiny, and their
                    # uses are naturally ordered within the softmax chain).
                    psmall = pss_pool.tile([128, 512], F32, tag="psmall")
                    scores = psmall[0:32, 0:16]
                    nc.tensor.matmul(scores, E32[:], s4[:], start=True, stop=True)

                    # |scores| is O(1) (h in (-1,1), scaled by 1/sqrt(H)):
                    # skip the max-subtract, exp cannot overflow.
                    ex = wrk.tile([32, 16], F32, tag="ex")
                    esum = wrk.tile([32, 1], F32, tag="esum")
                    nc.scalar.activation(
                        ex[:], scores, AF.Exp, scale=1.0, accum_out=esum[:]
                    )
                    rcp = wrk.tile([32, 1], F32, tag="rcp")
                    nc.vector.reciprocal(rcp[:], esum[:])
                    # normalize + cast in one op, transpose in bf16
                    nc.vector.tensor_scalar_mul(wsq[:, 0:16], ex[:], rcp[:])
                    wT = wrk.tile([32, 32], BF16, tag="wT")
                    nc.vector.transpose(wT[:], wsq[:])
                    pwx = psmall[:, 128:160]
                    nc.tensor.matmul(pwx, E16[:], wT[0:16, :], start=True, stop=True)
                    # block-diagonal expansion, split so the first P matmuls
                    # start after half the broadcast multiply
                    masked = wrk.tile([128, 512], BF16, tag="masked")
                    for gh in range(2):
                        nc.vector.tensor_mul(
                            masked[:, 256 * gh : 256 * (gh + 1)].rearrange(
                                "p (b n) -> p b n", b=8
                            ),
                            pwx.rearrange("p (o n) -> p o n", o=1).broadcast_to(
                                [128, 8, 32]
                            ),
                            M32R[:, 256 * gh : 256 * (gh + 1)].rearrange(
                                "p (b n) -> p b n", b=8
                            ),
                        )

                    # -- attention contribution for both strips first, so the
                    # PSUM->SBUF staging copies overlap with the P matmuls
                    for r in range(2):
                        for g in range(G):
                            for j in range(4):
                                nc.tensor.matmul(
                                    strips[r][32 * j : 32 * (j + 1), :],
                                    masked[:, g * 128 + 32 * j : g * 128 + 32 * j + 32],
                                    P_sb[:, g * H4 + j * 1024 + r * 512 : g * H4 + j * 1024 + r * 512 + 512],
                                    start=False,
                                    stop=(g == G - 1),
                                    skip_group_check=True,
                                    tile_position=(0, 32 * j),
                                )
                    # PSUM -> SBUF staging for the PE transpose: r=0 on ScalarE
                    # (faster PSUM port), r=1 on VectorE, running in parallel
                    pats = []
                    for r in range(2):
                        sg = wrk.tile([128, 512], F32, tag=f"sg{r}")
                        (nc.scalar.copy if r == 0 else nc.vector.tensor_copy)(
                            sg[:], strips[r][:]
                        )
                        pat = paT_pool.tile([128, 512], F32, tag=f"pat{r}")
                        for q in range(4):
                            nc.tensor.matmul(
                                pat[:, 128 * q : 128 * (q + 1)],
                                sg[:, 128 * q : 128 * (q + 1)],
                                eye[:],
                                is_transpose=True,
                                start=(q == 0),
                                stop=(q == 3),
                            )
                        pats.append(pat)

                    for r in range(2):
                        pat = pats[r]
                        # one tanh for all four gates: the host pre-scales the
                        # g-gate weight columns by 2, so tanh(a*0.5) yields
                        # tanh(a_ifo/2) for i/f/o and tanh(a_g) for g. The
                        # sigmoid affine 0.5*(1+t) is folded into the cell
                        # math via scalar_tensor_tensor with cT keeping 2c.
                        act = wrk.tile([128, 512], F32, tag=f"act{r}")
                        nc.scalar.activation(act[:], pat[:], AF.Tanh, scale=0.5)
                        ti_v = q4(act[:])[:, :, 0:32]
                        tf_v = q4(act[:])[:, :, 32:64]
                        to_v = q4(act[:])[:, :, 64:96]
                        g_v = q4(act[:])[:, :, 96:128]
                        cview = cT[:, 128 * r : 128 * (r + 1)].rearrange(
                            "p (q n) -> p q n", q=4
                        )
                        # 2ig = (ti + 1) * g ; 4fc = (tf + 1) * C2
                        ig = wrk.tile([128, 128], F32, tag=f"ig{r}")
                        nc.vector.scalar_tensor_tensor(
                            q4(ig[:]), ti_v, 1.0, g_v, ALU.add, ALU.mult
                        )
                        fc = wrk.tile([128, 128], F32, tag=f"fc{r}")
                        nc.vector.scalar_tensor_tensor(
                            q4(fc[:]), tf_v, 1.0, cview, ALU.add, ALU.mult
                        )
                        # C2' = 2(fc + ig) = 4fc * 0.5 + 2ig
                        nc.vector.scalar_tensor_tensor(
                            cview, q4(fc[:]), 0.5, q4(ig[:]), ALU.mult, ALU.add
                        )
                        tch = wrk.tile([128, 128], F32, tag=f"tch{r}")
                        nc.scalar.activation(
                            tch[:], cT[:, 128 * r : 128 * (r + 1)], AF.Tanh, scale=0.5
                        )
                        # uTh <- 2h = (to + 1) * tanh(c') directly (bf16 cast
                        # in the same op — this is the cross-step critical path)
                        nc.vector.scalar_tensor_tensor(
                            uTh[:, 128 * r : 128 * (r + 1)].rearrange(
                                "p (q n) -> p q n", q=4
                            ),
                            to_v, 1.0,
                            tch[:].rearrange("p (q n) -> p q n", q=4),
                            ALU.add, ALU.mult,
                        )
                        # separate bf16 2h copy for the output DMA so the DMA
                        # read never back-pressures the next uTh write (host
                        # casts to f32 and applies the 1/2)
                        h2 = wrk.tile([128, 128], BF16, tag=f"h2{r}")
                        nc.vector.scalar_tensor_tensor(
                            h2[:].rearrange("p (q n) -> p q n", q=4),
                            to_v, 1.0,
                            tch[:].rearrange("p (q n) -> p q n", q=4),
                            ALU.add, ALU.mult,
                        )
                        nc.sync.dma_start(
                            outT[t, 512 * r : 512 * (r + 1), :].rearrange(
                                "(q p) n -> p q n", p=128
                            ),
                            h2[:].rearrange("p (q n) -> p q n", q=4),
                        )
    nc.compile()
    return nc


def _prep_shards(inputs):
    x = np.asarray(inputs["x"], np.float32)
    A = np.asarray(inputs["A"], np.float32)
    Wx = np.asarray(inputs["Wx"], np.float32)
    Wh = np.asarray(inputs["Wh"], np.float32)
    Wattn = np.asarray(inputs["Wattn"], np.float32)
    b = np.asarray(inputs["b"], np.float32)

    # The kernel keeps uTh = 2h (so Wh absorbs a 1/2) and evaluates all four
    # gates with a single tanh(a/2): the g-gate weight columns absorb a 2.
    gscale = np.ones((1, H4), np.float32)
    gscale[0, 3 * H :] = 2.0
    Wx_bf = np.ascontiguousarray((Wx * gscale).astype(BF))
    Wh_bf = np.ascontiguousarray((Wh * 0.5 * gscale).astype(BF))
    Wa_bf = np.ascontiguousarray((Wattn * gscale).astype(BF))
    b_bf = np.ascontiguousarray((b.reshape(1, H4) * gscale).astype(BF))

    in_maps = []
    for i in range(NCORES):
        ns = slice(NL * i, NL * (i + 1))
        # xTs[t, p, c, n] = x[n, t, 128 c + p]
        xTs = x[ns].transpose(1, 2, 0).reshape(T, HC, 128, NL).transpose(0, 2, 1, 3)
        xTs = xTs.reshape(T, 128, HC * NL)
        Asc = A[ns].reshape(NL, H, 16).transpose(1, 2, 0).reshape(H, 512)
        in_maps.append(
            {
                "xTs": np.ascontiguousarray(xTs.astype(BF)),
                "Asc": np.ascontiguousarray(Asc.astype(BF)),
                "Wx": Wx_bf,
                "Wh": Wh_bf,
                "Wattn": Wa_bf,
                "brow": b_bf,
            }
        )
    return in_maps


def _get_nc():
    global _built
    if _built is None:
        _built = _build_nc()
    return _built


def _run(inputs, **kwargs):
    nc = _get_nc()
    in_maps = _prep_shards(inputs)
    res = bass_utils.run_bass_kernel_spmd(
        nc, in_maps, core_ids=list(range(NCORES)), **kwargs
    )
    out = np.empty((N, T, H), np.float32)
    for i in range(NCORES):
        # outT holds 2h in bf16 — cast up and halve on the host
        o = res.results[i]["outT"].astype(np.float32) * 0.5
        out[NL * i : NL * (i + 1)] = o.transpose(2, 0, 1)
    return out, res


def kernel(**inputs):
    out, _ = _run(inputs)
    return out


# revision 32
# speedup vs baseline: 1.2985x; 1.0367x over previous
"""Trainium2 Bass kernel for nn_CaptioningRNN (attention LSTM over T=64).

Data-parallel over the batch: N=256 samples split across 8 NeuronCores
(32 samples/core), weights replicated, no collectives.

Per-core design (v3 — fully fused step loop):
  - No xproj prepass: x_t @ Wx accumulates into the same PSUM strips as
    h @ Wh and the attention term, so there is no DRAM scratch round trip
    and the TensorEngine stays dense (HAM stays warm).
  - Single ACT table set (exp_and_others): sigmoid is computed as
    0.5*(1+tanh(x/2)) so the per-step Exp (softmax) and Tanh (gates) never
    force an activation-table reload.
  - P precompute: P[n,k,:] = A[n,:,k] @ Wattn + b once; since softmax
    weights sum to 1, folding b into P applies the bias exactly.
  - Per step: scores via cross-sample matmuls + masked diag reduce;
    softmax (no max-subtract — scores are O(1) bounded); w expanded to the
    (k, n_g) block-diagonal stationary via one-hot matmul + broadcast mask
    mul; gates = h@Wh + x_t@Wx + sum_k w_k P_k in 2 column-tiled PSUM
    strips; strips transposed on PE; cell math in h-on-partition space.
  - Output written transposed [t, h, n]; host reassembles to (N, T, H).
"""

import numpy as np
import ml_dtypes

import concourse.bacc as bacc
import concourse.mybir as mybir
from concourse import bass_utils
from concourse.tile import TileContext

F32, BF16 = mybir.dt.float32, mybir.dt.bfloat16
AF = mybir.ActivationFunctionType
ALU = mybir.AluOpType
AX = mybir.AxisListType
BF = ml_dtypes.bfloat16

N, T, D, H = 256, 64, 1024, 1024
NCORES = 8
NL = N // NCORES          # 32 samples per core
HC = 8                    # 128-row chunks of D/H
G, GS = 4, 8              # sample groups of 8 (for the (k, n_g) 128-partition layout)
H4 = 4 * H                # 4096 gate columns

_built = None


def _consts():
    # E16[k', k] one-hot: expands wT rows onto the 128-partition (k, n_g) axis.
    e16 = np.zeros((16, 128), dtype=BF)
    for k in range(16):
        e16[k, 8 * k : 8 * k + 8] = 1
    # M32R[p, 128 g + 32 rep + m] = (m % 8 == p % 8) & (m // 8 == g):
    # block-diagonal mask producing masked = w[m, k(p)] only for group-g
    # samples, replicated over the 4 column-tile strips.
    p = np.arange(128)[:, None]
    m = np.arange(32)[None, :]
    m32r = np.zeros((128, 512), dtype=BF)
    for g in range(4):
        blk = ((m % 8 == p % 8) & (m // 8 == g)).astype(BF)
        for rep in range(4):
            m32r[:, 128 * g + 32 * rep : 128 * g + 32 * rep + 32] = blk
    # Mdiag4[32 j + n, 32 k + n'] = (n == n') / 32: extracts the diagonal of
    # the col-tiled cross-sample score partials and applies the 1/sqrt(H)
    # softmax scale (same pattern for each of the 4 partition tiles).
    md = np.zeros((32, 512), dtype=np.float32)
    n_ = np.arange(32)
    for k in range(16):
        md[n_, 32 * k + n_] = 1.0 / 32.0
    # uTh holds 2h, so the score scale absorbs an extra 1/2 (1/64 total)
    md4 = np.tile(md, (4, 1)) * 0.5
    # E32[p, m] = (p % 32 == m): sums the 4 col-tiled score partials (f32
    # stationary so the tiny N=16 matmul needs no cast of its moving operand).
    e32 = np.zeros((128, 32), dtype=np.float32)
    e32[np.arange(128), np.arange(128) % 32] = 1
    # row-0 selector for the rank-1 bias accumulation into P.
    er0 = np.zeros((128, 128), dtype=BF)
    er0[0, :] = 1
    return e16, m32r, md4, e32, er0


def _build_nc(t_steps=T):
    nc = bacc.Bacc(trn_type="TRN2", target_bir_lowering=False, debug=False)

    # xTs[t, p, c, n] = x[n, t, 128 c + p] (bf16) — one contiguous DMA/step
    ap_xTs = nc.dram_tensor("xTs", [T, 128, HC * NL], BF16, kind="ExternalInput").ap()
    ap_Asc = nc.dram_tensor("Asc", [H, 512], BF16, kind="ExternalInput").ap()
    ap_Wx = nc.dram_tensor("Wx", [D, H4], BF16, kind="ExternalInput").ap()
    ap_Wh = nc.dram_tensor("Wh", [H, H4], BF16, kind="ExternalInput").ap()
    ap_Wattn = nc.dram_tensor("Wattn", [H, H4], BF16, kind="ExternalInput").ap()
    ap_brow = nc.dram_tensor("brow", [1, H4], BF16, kind="ExternalInput").ap()
    outT = nc.dram_tensor("outT", [T, H, NL], BF16, kind="ExternalOutput").ap()

    e16_np, m32r_np, md4_np, e32_np, er0_np = _consts()
    eye_d = nc.inline_tensor(np.eye(128, dtype=BF), "c_eye")
    e16_d = nc.inline_tensor(e16_np, "c_e16")
    m32r_d = nc.inline_tensor(m32r_np, "c_m32r")
    md4_d = nc.inline_tensor(md4_np, "c_mdiag4")
    e32_d = nc.inline_tensor(e32_np, "c_e32")
    er0_d = nc.inline_tensor(er0_np, "c_er0")

    q4 = lambda ap: ap.rearrange("p (q c) -> p q c", q=4)

    with TileContext(nc) as tc:
        with tc.tile_pool(name="pers", bufs=1) as pers:
            Wh_sb = pers.tile([128, HC * H4], BF16, tag="Wh")
            Wx_sb = pers.tile([128, HC * H4], BF16, tag="Wx")
            Asc_sb = pers.tile([128, HC * 512], BF16, tag="Asc")
            P_sb = pers.tile([128, G * H4], BF16, tag="P")
            uTh = pers.tile([128, HC * NL], BF16, tag="uTh")
            cT = pers.tile([128, 256], F32, tag="cT")
            eye = pers.tile([128, 128], BF16, tag="eye")
            E16 = pers.tile([16, 128], BF16, tag="E16")
            M32R = pers.tile([128, 512], BF16, tag="M32R")
            Mdiag4 = pers.tile([128, 512], F32, tag="Mdiag4")
            E32 = pers.tile([128, 32], F32, tag="E32")
            ER0 = pers.tile([128, 128], BF16, tag="ER0")
            wsq = pers.tile([32, 32], BF16, tag="wsq")

            # Asc + the first Wattn blocks are what phase B needs — issue
            # those DMAs before the big Wh/Wx loads so the P matmuls can
            # start early (the Wh/Wx weights are only needed at step 0).
            for c in range(HC):
                nc.sync.dma_start(
                    Asc_sb[:, c * 512 : (c + 1) * 512],
                    ap_Asc[128 * c : 128 * (c + 1), :],
                )
            nc.sync.dma_start(eye[:], eye_d.ap()[:])
            nc.sync.dma_start(E16[:], e16_d.ap()[:])
            nc.sync.dma_start(M32R[:], m32r_d.ap()[:])
            nc.sync.dma_start(Mdiag4[:], md4_d.ap()[:])
            nc.sync.dma_start(E32[:], e32_d.ap()[:])
            nc.sync.dma_start(ER0[:], er0_d.ap()[:])
            nc.gpsimd.memset(wsq[:], 0.0)

            # ------------- phase B: P precompute (+bias) + h0/c0 init -------------
            with tc.tile_pool(name="php1", bufs=1) as php1, \
                 tc.tile_pool(name="php", bufs=2) as php, \
                 tc.tile_pool(name="psP", bufs=2, space="PSUM") as psP:
                for c in range(HC):
                    h0s = php.tile([128, 32], F32, tag="h0s")
                    nc.vector.tensor_reduce(
                        h0s[:],
                        Asc_sb[:, c * 512 : (c + 1) * 512].rearrange(
                            "p (k n) -> p n k", k=16
                        ),
                        axis=AX.X,
                        op=ALU.add,
                    )
                    # cT holds C2 = 2*c and uTh holds 2*h throughout (the cell
                    # update keeps the doubled scale; tanh reads with scale=0.5
                    # and the host pre-scales Wh by 1/2)
                    nc.vector.tensor_scalar_mul(
                        cT[:, 32 * c : 32 * (c + 1)], h0s[:], 1.0 / 8.0
                    )
                    nc.vector.tensor_scalar_mul(
                        uTh[:, 32 * c : 32 * (c + 1)], h0s[:], 1.0 / 8.0
                    )
                # contiguous staging of the group-selected A columns so the
                # matmul stationary operand has a single free dim
                Ag = php1.tile([128, G * HC * 128], BF16, tag="Ag")
                for g in range(G):
                    for c in range(HC):
                        nc.vector.tensor_copy(
                            Ag[:, (g * HC + c) * 128 : (g * HC + c) * 128 + 128],
                            Asc_sb[:, c * 512 : (c + 1) * 512].rearrange(
                                "p (k n) -> p k n", k=16
                            )[:, :, GS * g : GS * (g + 1)],
                        )
                bmv = php1.tile([128, 512], BF16, tag="bmv")
                nc.gpsimd.memset(bmv[:], 0.0)
                for blk in range(8):
                    Wab = php.tile([128, HC * 512], BF16, tag="Wab")
                    for c in range(HC):
                        nc.sync.dma_start(
                            Wab[:, c * 512 : (c + 1) * 512],
                            ap_Wattn[128 * c : 128 * (c + 1), 512 * blk : 512 * (blk + 1)],
                        )
                    if blk >= 2:
                        # big Wh/Wx loads spread between the Wattn blocks so
                        # they never monopolize the DMA queues ahead of the
                        # phase-B weights (they are only needed at step 0)
                        for ci in range(3 * (blk - 2), min(3 * (blk - 1), 16)):
                            w_sb, ap_w = ((Wh_sb, ap_Wh), (Wx_sb, ap_Wx))[ci % 2]
                            c = ci // 2
                            nc.sync.dma_start(
                                w_sb[:, c * H4 : (c + 1) * H4],
                                ap_w[128 * c : 128 * (c + 1), :],
                            )
                    bsl = php.tile([1, 512], BF16, tag="bsl")
                    nc.sync.dma_start(bsl[:], ap_brow[0:1, 512 * blk : 512 * (blk + 1)])
                    nc.vector.tensor_copy(bmv[0:1, :], bsl[:])
                    for g in range(G):
                        psp = psP.tile([128, 512], F32, tag="psp")
                        for c in range(HC):
                            nc.tensor.matmul(
                                psp[:],
                                Ag[:, (g * HC + c) * 128 : (g * HC + c) * 128 + 128],
                                Wab[:, c * 512 : (c + 1) * 512],
                                start=(c == 0),
                                stop=False,
                            )
                        # rank-1 accumulation: adds b[blk cols] to every row
                        # (softmax weights sum to 1, so this applies +b exactly)
                        nc.tensor.matmul(
                            psp[:], ER0[:], bmv[:], start=False, stop=True
                        )
                        nc.vector.tensor_copy(
                            P_sb[:, g * H4 + 512 * blk : g * H4 + 512 * (blk + 1)],
                            psp[:],
                        )

            # ---------------------- phase C: recurrence ----------------------
            with tc.tile_pool(name="wrk", bufs=2) as wrk, \
                 tc.tile_pool(name="xio", bufs=3) as xio, \
                 tc.tile_pool(name="psc", bufs=1, space="PSUM") as psc_pool, \
                 tc.tile_pool(name="pss", bufs=1, space="PSUM") as pss_pool, \
                 tc.tile_pool(name="pstr", bufs=2, space="PSUM") as pstr_pool, \
                 tc.tile_pool(name="paT", bufs=1, space="PSUM") as paT_pool:
                for t in range(t_steps):
                    # prefetched x_t slice (bf16, contiguous per partition)
                    xt = xio.tile([128, HC * NL], BF16, tag="xt", name=f"xt_{t}")
                    nc.sync.dma_start(xt[:], ap_xTs[t])

                    strips = [
                        pstr_pool.tile([128, 512], F32, tag=f"strip{r}",
                                       name=f"strip{r}_{t}")
                        for r in range(2)
                    ]

                    # -- x_t @ Wx first: the only PE work independent of h, so
                    # it covers the previous step's cell-update tail and keeps
                    # the PE dense (HAM stays warm).
                    for c in range(HC):
                        for r in range(2):
                            for j in range(4):
                                nc.tensor.matmul(
                                    strips[r][32 * j : 32 * (j + 1), :],
                                    xt[:, 32 * c : 32 * (c + 1)],
                                    Wx_sb[:, c * H4 + j * 1024 + r * 512 : c * H4 + j * 1024 + r * 512 + 512],
                                    start=(c == 0),
                                    stop=False,
                                    skip_group_check=True,
                                    tile_position=(0, 32 * j),
                                )

                    # -- scores: cross-sample products, col-tiled 4-wide (tile
                    # j accumulates h-chunks 2j, 2j+1 into partition rows 32j+).
                    # Tiles j=0,1 only need the r=0 half of uTh, so they run
                    # as soon as the previous step's first half-update lands.
                    psc4 = psc_pool.tile([128, 512], F32, tag="psc4")
                    def psc_half(half):
                        # cc-outer so the two tiles of each half run
                        # concurrently in their column strips
                        for cc in range(2):
                            for j in (0, 1) if half == 0 else (2, 3):
                                c = 2 * j + cc
                                nc.tensor.matmul(
                                    psc4[32 * j : 32 * (j + 1), :],
                                    uTh[:, 32 * c : 32 * (c + 1)],
                                    Asc_sb[:, c * 512 : (c + 1) * 512],
                                    start=(cc == 0),
                                    stop=(cc == 1),
                                    skip_group_check=True,
                                    tile_position=(0, 32 * j),
                                )
                    def wh_half(half):
                        for c in range(4 * half, 4 * half + 4):
                            for r in range(2):
                                for j in range(4):
                                    nc.tensor.matmul(
                                        strips[r][32 * j : 32 * (j + 1), :],
                                        uTh[:, 32 * c : 32 * (c + 1)],
                                        Wh_sb[:, c * H4 + j * 1024 + r * 512 : c * H4 + j * 1024 + r * 512 + 512],
                                        start=False,
                                        stop=False,
                                        skip_group_check=True,
                                        tile_position=(0, 32 * j),
                                    )
                    psc_half(0)
                    wh_half(0)
                    psc_half(1)
                    wh_half(1)

                    # diag extract + partial reduce (DVE; cost scales with the
                    # free dim, so one full-width op each — never row-split)
                    scm = wrk.tile([128, 512], F32, tag="scm")
                    s4 = wrk.tile([128, 16], F32, tag="s4")
                    nc.vector.tensor_mul(scm[:], psc4[:], Mdiag4[:])
                    nc.vector.tensor_reduce(
                        s4[:],
                        scm[:].rearrange("p (k n) -> p k n", k=16),
                        axis=AX.X,
                        op=ALU.add,
                    )
                    # sum the 4 col-tile partials with a tiny f32 matmul.
                    # scores and pwx share one PSUM bank (both tiny, and their
                    # uses are naturally ordered within the softmax chain).
                    psmall = pss_pool.tile([128, 512], F32, tag="psmall")
                    scores = psmall[0:32, 0:16]
                    nc.tensor.matmul(scores, E32[:], s4[:], start=True, stop=True)

                    # |scores| is O(1) (h in (-1,1), scaled by 1/sqrt(H)):
                    # skip the max-subtract, exp cannot overflow.
                    ex = wrk.tile([32, 16], F32, tag="ex")
                    esum = wrk.tile([32, 1], F32, tag="esum")
                    nc.scalar.activation(
                        ex[:], scores, AF.Exp, scale=1.0, accum_out=esum[:]
                    )
                    rcp = wrk.tile([32, 1], F32, tag="rcp")
                    nc.vector.reciprocal(rcp[:], esum[:])
                    # normalize + cast in one op, transpose in bf16
                    nc.vector.tensor_scalar_mul(wsq[:, 0:16], ex[:], rcp[:])
                    wT = wrk.tile([32, 32], BF16, tag="wT")
                    nc.vector.transpose(wT[:], wsq[:])
                    pwx = psmall[:, 128:160]
                    nc.tensor.matmul(pwx, E16[:], wT[0:16, :], start=True, stop=True)
                    # block-diagonal expansion, split so the first P matmuls
                    # start after half the broadcast multiply
                    masked = wrk.tile([128, 512], BF16, tag="masked")
                    for gh in range(2):
                        nc.vector.tensor_mul(
                            masked[:, 256 * gh : 256 * (gh + 1)].rearrange(
                                "p (b n) -> p b n", b=8
                            ),
                            pwx.rearrange("p (o n) -> p o n", o=1).broadcast_to(
                                [128, 8, 32]
                            ),
                            M32R[:, 256 * gh : 256 * (gh + 1)].rearrange(
                                "p (b n) -> p b n", b=8
                            ),
                        )

                    # -- attention contribution for both strips first, so the
                    # PSUM->SBUF staging copies overlap with the P matmuls
                    for r in range(2):
                        for g in range(G):
                            for j in range(4):
                                nc.tensor.matmul(
                                    strips[r][32 * j : 32 * (j + 1), :],
                                    masked[:, g * 128 + 32 * j : g * 128 + 32 * j + 32],
                                    P_sb[:, g * H4 + j * 1024 + r * 512 : g * H4 + j * 1024 + r * 512 + 512],
                                    start=False,
                                    stop=(g == G - 1),
                                    skip_group_check=True,
                                    tile_position=(0, 32 * j),
                                )
                    # PSUM -> SBUF staging for the PE transpose: r=0 on ScalarE
                    # (faster PSUM port), r=1 on VectorE, running in parallel
                    pats = []
                    for r in range(2):
                        sg = wrk.tile([128, 512], BF16, tag=f"sg{r}")
                        (nc.scalar.copy if r == 0 else nc.vector.tensor_copy)(
                            sg[:], strips[r][:]
                        )
                        pat = paT_pool.tile([128, 512], BF16, tag=f"pat{r}")
                        for q in range(4):
                            nc.tensor.matmul(
                                pat[:, 128 * q : 128 * (q + 1)],
                                sg[:, 128 * q : 128 * (q + 1)],
                                eye[:],
                                is_transpose=True,
                                start=(q == 0),
                                stop=(q == 3),
                            )
                        pats.append(pat)

                    for r in range(2):
                        pat = pats[r]
                        # one tanh for all four gates: the host pre-scales the
                        # g-gate weight columns by 2, so tanh(a*0.5) yields
                        # tanh(a_ifo/2) for i/f/o and tanh(a_g) for g. The
                        # sigmoid affine 0.5*(1+t) is folded into the cell
                        # math via scalar_tensor_tensor with cT keeping 2c.
                        act = wrk.tile([128, 512], F32, tag=f"act{r}")
                        nc.scalar.activation(act[:], pat[:], AF.Tanh, scale=0.5)
                        ti_v = q4(act[:])[:, :, 0:32]
                        tf_v = q4(act[:])[:, :, 32:64]
                        to_v = q4(act[:])[:, :, 64:96]
                        g_v = q4(act[:])[:, :, 96:128]
                        cview = cT[:, 128 * r : 128 * (r + 1)].rearrange(
                            "p (q n) -> p q n", q=4
                        )
                        # 2ig = (ti + 1) * g ; 4fc = (tf + 1) * C2
                        ig = wrk.tile([128, 128], F32, tag=f"ig{r}")
                        nc.vector.scalar_tensor_tensor(
                            q4(ig[:]), ti_v, 1.0, g_v, ALU.add, ALU.mult
                        )
                        fc = wrk.tile([128, 128], F32, tag=f"fc{r}")
                        nc.vector.scalar_tensor_tensor(
                            q4(fc[:]), tf_v, 1.0, cview, ALU.add, ALU.mult
                        )
                        # C2' = 2(fc + ig) = 4fc * 0.5 + 2ig
                        nc.vector.scalar_tensor_tensor(
                            cview, q4(fc[:]), 0.5, q4(ig[:]), ALU.mult, ALU.add
                        )
                        tch = wrk.tile([128, 128], F32, tag=f"tch{r}")
                        nc.scalar.activation(
                            tch[:], cT[:, 128 * r : 128 * (r + 1)], AF.Tanh, scale=0.5
                        )
                        # uTh <- 2h = (to + 1) * tanh(c') directly (bf16 cast
                        # in the same op — this is the cross-step critical path)
                        nc.vector.scalar_tensor_tensor(
                            uTh[:, 128 * r : 128 * (r + 1)].rearrange(
                                "p (q n) -> p q n", q=4
                            ),
                            to_v, 1.0,
                            tch[:].rearrange("p (q n) -> p q n", q=4),
                            ALU.add, ALU.mult,
                        )
                        # separate bf16 2h copy for the output DMA so the DMA
                        # read never back-pressures the next uTh write (host
                        # casts to f32 and applies the 1/2)
                        h2 = wrk.tile([128, 128], BF16, tag=f"h2{r}")
                        nc.vector.scalar_tensor_tensor(
                            h2[:].rearrange("p (q n) -> p q n", q=4),
                            to_v, 1.0,
                            tch[:].rearrange("p (q n) -> p q n", q=4),
                            ALU.add, ALU.mult,
                        )
                        nc.sync.dma_start(
                            outT[t, 512 * r : 512 * (r + 1), :].rearrange(
                                "(q p) n -> p q n", p=128
                            ),
                            h2[:].rearrange("p (q n) -> p q n", q=4),
                        )
    nc.compile()
    return nc


def _prep_shards(inputs):
    x = np.asarray(inputs["x"], np.float32)
    A = np.asarray(inputs["A"], np.float32)
    Wx = np.asarray(inputs["Wx"], np.float32)
    Wh = np.asarray(inputs["Wh"], np.float32)
    Wattn = np.asarray(inputs["Wattn"], np.float32)
    b = np.asarray(inputs["b"], np.float32)

    # The kernel keeps uTh = 2h (so Wh absorbs a 1/2) and evaluates all four
    # gates with a single tanh(a/2): the g-gate weight columns absorb a 2.
    gscale = np.ones((1, H4), np.float32)
    gscale[0, 3 * H :] = 2.0
    Wx_bf = np.ascontiguousarray((Wx * gscale).astype(BF))
    Wh_bf = np.ascontiguousarray((Wh * 0.5 * gscale).astype(BF))
    Wa_bf = np.ascontiguousarray((Wattn * gscale).astype(BF))
    b_bf = np.ascontiguousarray((b.reshape(1, H4) * gscale).astype(BF))

    in_maps = []
    for i in range(NCORES):
        ns = slice(NL * i, NL * (i + 1))
        # xTs[t, p, c, n] = x[n, t, 128 c + p]
        xTs = x[ns].transpose(1, 2, 0).reshape(T, HC, 128, NL).transpose(0, 2, 1, 3)
        xTs = xTs.reshape(T, 128, HC * NL)
        Asc = A[ns].reshape(NL, H, 16).transpose(1, 2, 0).reshape(H, 512)
        in_maps.append(
            {
                "xTs": np.ascontiguousarray(xTs.astype(BF)),
                "Asc": np.ascontiguousarray(Asc.astype(BF)),
                "Wx": Wx_bf,
                "Wh": Wh_bf,
                "Wattn": Wa_bf,
                "brow": b_bf,
            }
        )
    return in_maps


def _get_nc():
    global _built
    if _built is None:
        _built = _build_nc()
    return _built


def _run(inputs, **kwargs):
    nc = _get_nc()
    in_maps = _prep_shards(inputs)
    res = bass_utils.run_bass_kernel_spmd(
        nc, in_maps, core_ids=list(range(NCORES)), **kwargs
    )
    out = np.empty((N, T, H), np.float32)
    for i in range(NCORES):
        # outT holds 2h in bf16 — cast up and halve on the host
        o = res.results[i]["outT"].astype(np.float32) * 0.5
        out[NL * i : NL * (i + 1)] = o.transpose(2, 0, 1)
    return out, res


def kernel(**inputs):
    out, _ = _run(inputs)
    return out
